# revision 2
# baseline (speedup 1.0000x reference)
"""Trainium2 Bass kernel for nn_CumsumInterpolationEmbedding.

    E = LayerNorm(cumsum(cumsum_embedding, axis=0)) * gamma + beta
        + [alpha, 1-alpha] @ interpolation_embedding     # alpha_n = (N-1-n)/(N-1)
    out[b, l, :] = E[index_tensor[b, l], :]              # [4096, 200, 128] f32

Data-parallel over the 4096 batch across 8 NeuronCores; each core runs an
identical program on its own 1/8 of the index stream (102,400 lookups).

Per-core device program:
  prologue — build E [1024, 128] once (PE cumsum via triangular/ones matmuls,
    DVE bn_stats/bn_aggr layernorm, ACT rsqrt, K=3 matmul for interpolation +
    beta), cast it to bf16 and park it in a DRAM scratch tensor.
  main loop — J chunks of C=3200 lookups, three pipelined stages:
    * gpsimd dma_gather: 3200 descriptors of 256 B (bf16) each, table rows ->
      SBUF tile [128, G=25, 128] (row i of the chunk lands on partition i%128),
      spread round-robin over 4 SWDGE queues (the ucode max; each queue is
      served by its own gpsimd core pair).
    * upcast bf16 -> f32 on the otherwise idle DVE + ACT engines (half each).
    * HWDGE writeout of the f32 tile to the per-core contiguous output slice.

Why this shape: the gather is Q7 descriptor-EMISSION-bound (~1.8 ns/row
aggregate over 4 queues, ~185 us/pass, independent of descriptor size), and
gather + writeout share the 16-SDMA-engine pool (~360-400 GB/s). Keeping the
parked table in bf16 halves the gather's share of SDMA bytes (26 MB vs 52 MB
per pass), and the fine chunk grain (C=3200) with deep buffering (12 gather
buffers, 6 writeout buffers) overlaps the emission with the SDMA drain;
measured ~254 us/pass vs ~470 us for the f32 coarse-grained two-pass. Table
quantization to bf16 adds ~2e-3 relative error (gate is 2e-2).

The host pre-permutes each core's index stream so that partition p of chunk j
holds output rows jC + p*G .. jC + p*G + G-1; the writeout is then 128
descriptors of G*512 B contiguous DRAM each, and the gathered rows land in
exactly original output order. Host work is only casting/reordering the int
indices (no arithmetic on the data path).
"""

import contextlib

import numpy as np

import concourse.bacc as bacc
import concourse.bass as bass
import concourse.mybir as mybir
from concourse._compat import get_trn_type
from concourse.bass_utils import run_bass_kernel_spmd
from concourse.library_config import mlp

F32 = mybir.dt.float32
BF16 = mybir.dt.bfloat16
I16 = mybir.dt.int16
AF = mybir.ActivationFunctionType
ALU = mybir.AluOpType

N_CORES = 8
N = 1024            # embedding table rows
D = 128             # embedding dim
EPS = 1e-5
B, L = 4096, 200
R = B * L // N_CORES  # 102400 output rows per core
G = 25                # output rows per partition per gather chunk
C = 128 * G           # 3200 indices per gather chunk
NB = 12               # bf16 gather buffers in flight
NW = 6                # f32 upcast/writeout buffers in flight
NQ = 4                # SWDGE queues (ucode max); chunk j uses queue j % NQ
TC = N // 128         # 8 table chunks
HALF = (G * D) // 2   # upcast split point (DVE first half, ACT second)


def build_nc(rows: int = R, reps: int = 1) -> bass.Bass:
    """reps > 1 repeats the main gather/upcast/writeout loop (idempotent
    rewrites of the same output) so device time can be measured as a slope."""
    J = rows // C
    assert J * C == rows and rows % 16 == 0
    assert J % NQ == 0
    JJ = J * reps
    IXC = rows // 16 // NQ  # idx columns per queue block

    nc = bacc.Bacc(get_trn_type() or "TRN2", num_swdge_queues=NQ)
    csemb = nc.dram_tensor("csemb", [N, D], F32, kind="ExternalInput")
    triu = nc.dram_tensor("triu", [128, 128], F32, kind="ExternalInput")
    onesm = nc.dram_tensor("onesm", [128, 128], F32, kind="ExternalInput")
    a3t = nc.dram_tensor("a3t", [3, N], F32, kind="ExternalInput")
    i3 = nc.dram_tensor("i3", [3, D], F32, kind="ExternalInput")
    gammab = nc.dram_tensor("gammab", [128, D], F32, kind="ExternalInput")
    idx16 = nc.dram_tensor("idx16", [128, IXC], I16, kind="ExternalInput")
    out = nc.dram_tensor("out", [rows, D], F32, kind="ExternalOutput")
    etab = nc.dram_tensor("etab", [N, D], BF16)

    with contextlib.ExitStack() as ctx:
        block = ctx.enter_context(nc.Block())
        cs_sb = ctx.enter_context(nc.sbuf_tensor("cs_sb", [128, TC, D], F32))
        triu_sb = ctx.enter_context(nc.sbuf_tensor("triu_sb", [128, 128], F32))
        ones_sb = ctx.enter_context(nc.sbuf_tensor("ones_sb", [128, 128], F32))
        a3t_sb = ctx.enter_context(nc.sbuf_tensor("a3t_sb", [3, N], F32))
        i3_sb = ctx.enter_context(nc.sbuf_tensor("i3_sb", [3, D], F32))
        gamma_sb = ctx.enter_context(nc.sbuf_tensor("gamma_sb", [128, D], F32))
        e_sb = ctx.enter_context(nc.sbuf_tensor("e_sb", [128, TC, D], F32))
        eb_sb = ctx.enter_context(nc.sbuf_tensor("eb_sb", [128, TC, D], BF16))
        stats_sb = ctx.enter_context(nc.sbuf_tensor("stats_sb", [128, TC, 6], F32))
        mv_sb = ctx.enter_context(nc.sbuf_tensor("mv_sb", [128, TC, 2], F32))
        rstd_sb = ctx.enter_context(nc.sbuf_tensor("rstd_sb", [128, TC], F32))
        eps_sb = ctx.enter_context(nc.sbuf_tensor("eps_sb", [128, 1], F32))
        idx_sb = ctx.enter_context(nc.sbuf_tensor("idx_sb", [128, IXC], I16))
        gbufs = [
            ctx.enter_context(nc.sbuf_tensor(f"gbuf{b}", [128, G * D], BF16))
            for b in range(NB)
        ]
        fbufs = [
            ctx.enter_context(nc.sbuf_tensor(f"fbuf{w}", [128, G * D], F32))
            for w in range(NW)
        ]
        ps_cs = ctx.enter_context(nc.psum_tensor("ps_cs", [128, TC, D], F32))
        ps_in = ctx.enter_context(nc.psum_tensor("ps_in", [128, TC, D], F32))

        s_in = ctx.enter_context(nc.semaphore("s_in"))
        s_ix = ctx.enter_context(nc.semaphore("s_ix"))
        s_pe = ctx.enter_context(nc.semaphore("s_pe"))
        s_pi = ctx.enter_context(nc.semaphore("s_pi"))
        s_mv = ctx.enter_context(nc.semaphore("s_mv"))
        s_rs = ctx.enter_context(nc.semaphore("s_rs"))
        s_e = ctx.enter_context(nc.semaphore("s_e"))
        s_eb = ctx.enter_context(nc.semaphore("s_eb"))
        s_et = ctx.enter_context(nc.semaphore("s_et"))
        s_eps = ctx.enter_context(nc.semaphore("s_eps"))
        s_dv = ctx.enter_context(nc.semaphore("s_dv"))
        g_sems = [ctx.enter_context(nc.semaphore(f"s_g{b}")) for b in range(NB)]
        # upcast-done (per f32 buffer): DVE and ACT each inc 1 per use
        u_sems = [ctx.enter_context(nc.semaphore(f"s_u{w}")) for w in range(NW)]
        w_sems = [ctx.enter_context(nc.semaphore(f"s_w{w}")) for w in range(NW)]

        @block.sync
        def _(sp: bass.BassEngine):
            sp.dma_start(cs_sb[:, :, :], csemb[:, :].rearrange("(c p) d -> p c d", p=128)).then_inc(s_in, 16)
            sp.dma_start(triu_sb[:, :], triu[:, :]).then_inc(s_in, 16)
            sp.dma_start(ones_sb[:, :], onesm[:, :]).then_inc(s_in, 16)
            sp.dma_start(a3t_sb[:, :], a3t[:, :]).then_inc(s_in, 16)
            sp.dma_start(i3_sb[:, :], i3[:, :]).then_inc(s_in, 16)
            sp.dma_start(gamma_sb[:, :], gammab[:, :]).then_inc(s_in, 16)
            sp.dma_start(idx_sb[:, :], idx16[:, :]).then_inc(s_ix, 16)

            # park the bf16 embedding table in DRAM for the gathers
            sp.wait_ge(s_eb, 1)
            sp.dma_start(
                etab[:, :].rearrange("(c p) d -> p c d", p=128), eb_sb[:, :, :]
            ).then_inc(s_et, 16)

            # writeouts: chunk j -> output rows [jC, (j+1)C), 128 descriptors
            # of G*512 B contiguous DRAM each, from the upcast f32 buffer
            for jj in range(JJ):
                j, w, r = jj % J, jj % NW, jj // NW
                sp.wait_ge(u_sems[w], 2 * (r + 1))
                sp.dma_start(
                    out[j * C : (j + 1) * C, :].rearrange("(p g) d -> p (g d)", p=128),
                    fbufs[w][:, :],
                ).then_inc(w_sems[w], 16)
            for w in range(min(NW, JJ)):
                sp.wait_ge(w_sems[w], 16 * ((JJ - 1 - w) // NW + 1))

        @block.tensor
        def _(pe: bass.BassEngine):
            pe.wait_ge(s_in, 96)
            # TRN2 fatal hazard: PE writing a PSUM bank while DVE reads the
            # same bank crashes the core. Interleave per chunk and gate each
            # chunk's matmuls on DVE being completely done with the previous
            # chunk (s_e) so PE-writes and DVE-reads never overlap in PSUM.
            for c in range(TC):
                if c >= 1:
                    pe.wait_ge(s_e, c)
                # cumsum: chunk c rows = ones @ chunks<c + triu @ chunk c,
                # accumulated in increasing row order (exact sequential
                # prefix sum)
                for cp in range(c + 1):
                    mm = pe.matmul(
                        ps_cs[:, c, :],
                        (triu_sb if cp == c else ones_sb)[:, :],
                        cs_sb[:, cp, :],
                        start=(cp == 0),
                        stop=(cp == c),
                    )
                    if cp == c:
                        mm.then_inc(s_pe, 1)
                # interpolation + beta: [alpha, 1-alpha, 1] @ [I0; I1; beta]
                pe.matmul(
                    ps_in[:, c, :],
                    a3t_sb[:, c * 128 : (c + 1) * 128],
                    i3_sb[:, :],
                    start=True,
                    stop=True,
                ).then_inc(s_pi, 1)

        @block.scalar
        def _(act: bass.BassEngine):
            act.wait_ge(s_eps, 1)
            for c in range(TC):
                act.wait_ge(s_mv, c + 1)
                act.activation(
                    out=rstd_sb[:, c : c + 1],
                    in_=mv_sb[:, c, 1:2],
                    func=AF.Sqrt,
                    bias=eps_sb[:, 0:1],
                    scale=1.0,
                ).then_inc(s_rs, 1)
            # upcast second half of each gathered chunk (ACT share)
            for jj in range(JJ):
                j, b, w, r = jj % J, jj % NB, jj % NW, jj // NW
                act.wait_ge(g_sems[b], 16 * (jj // NB + 1))
                if r >= 1:
                    act.wait_ge(w_sems[w], 16 * r)
                act.copy(
                    out=fbufs[w][:, HALF:],
                    in_=gbufs[b][:, HALF:],
                ).then_inc(u_sems[w], 1)

        @block.vector
        def _(dve: bass.BassEngine):
            # raw-bass DVE writes are posted: serialize each dependent step
            # with a sem (prologue-only, cost is negligible)
            dv_cnt = [0]

            def step(inst):
                dv_cnt[0] += 1
                inst.then_inc(s_dv, 1)
                dve.wait_ge(s_dv, dv_cnt[0])

            dve.memset(eps_sb[:, :], EPS).then_inc(s_eps, 1)
            for c in range(TC):
                dve.wait_ge(s_pe, c + 1)
                step(dve.bn_stats(out=stats_sb[:, c, :], in_=ps_cs[:, c, :]))
                dve.bn_aggr(out=mv_sb[:, c, :], in_=stats_sb[:, c, :]).then_inc(s_mv, 1)
                dve.wait_ge(s_rs, c + 1)
                step(dve.reciprocal(out=rstd_sb[:, c : c + 1], in_=rstd_sb[:, c : c + 1]))
                step(
                    dve.tensor_scalar(
                        out=e_sb[:, c, :],
                        in0=ps_cs[:, c, :],
                        scalar1=mv_sb[:, c, 0:1],
                        scalar2=rstd_sb[:, c : c + 1],
                        op0=ALU.subtract,
                        op1=ALU.mult,
                    )
                )
                step(
                    dve.tensor_tensor(
                        out=e_sb[:, c, :],
                        in0=e_sb[:, c, :],
                        in1=gamma_sb[:, :],
                        op=ALU.mult,
                    )
                )
                dve.wait_ge(s_pi, c + 1)
                dve.tensor_tensor(
                    out=e_sb[:, c, :], in0=e_sb[:, c, :], in1=ps_in[:, c, :], op=ALU.add
                ).then_inc(s_e, 1)
            # cast the finished table to bf16 for parking
            dve.tensor_copy(
                out=eb_sb[:, :, :].rearrange("p c d -> p (c d)"),
                in_=e_sb[:, :, :].rearrange("p c d -> p (c d)"),
            ).then_inc(s_eb, 1)
            # upcast first half of each gathered chunk (DVE share)
            for jj in range(JJ):
                j, b, w, r = jj % J, jj % NB, jj % NW, jj // NW
                dve.wait_ge(g_sems[b], 16 * (jj // NB + 1))
                if r >= 1:
                    dve.wait_ge(w_sems[w], 16 * r)
                dve.tensor_copy(
                    out=fbufs[w][:, :HALF],
                    in_=gbufs[b][:, :HALF],
                ).then_inc(u_sems[w], 1)

        @block.gpsimd
        def _(gp: bass.BassGpSimd):
            gp.load_library(mlp)
            gp.wait_ge(s_ix, 16)
            gp.wait_ge(s_et, 16)
            for jj in range(JJ):
                j, b = jj % J, jj % NB
                q = j % NQ
                if jj >= NB:
                    # gbuf reusable once both upcast halves of its prior use ran
                    w_prev = (jj - NB) % NW
                    gp.wait_ge(u_sems[w_prev], 2 * ((jj - NB) // NW + 1))
                gp.dma_gather(
                    gbufs[b][:, :].rearrange("p (g d) -> p g d", d=D),
                    etab[:, :],
                    idx_sb[:, (j // NQ) * (C // 16) : (j // NQ + 1) * (C // 16)],
                    C,
                    C,
                    D,
                    # single_packet concatenates all of an engine's descriptors
                    # into one packet; at C/16=200 descriptors per engine that
                    # blows the <=64-descriptor packet ceiling and kills the
                    # core. One packet per descriptor is required here.
                    single_packet=False,
                    queue_num=q,
                ).then_inc(g_sems[b], 16)

    nc.compile()
    return nc


_NC_CACHE: dict = {}


def _get_nc(rows: int, reps: int = 1) -> bass.Bass:
    if (rows, reps) not in _NC_CACHE:
        _NC_CACHE[(rows, reps)] = build_nc(rows, reps)
    return _NC_CACHE[(rows, reps)]


def prep_idx(flat: np.ndarray, rows: int) -> np.ndarray:
    """Permute + wrap one core's index stream for dma_gather.

    Gather flat position i = g*128 + p of chunk j must hold the index for
    output row jC + p*G + g, wrapped into the ucode's [16, n/16] layout
    (position i -> partition i%16, column i//16). Chunk j is processed by
    gpsimd core pair q = j % NQ, which reads indices from partitions
    [32q, 32q+32) only; chunk j's columns are packed at column offset
    (j // NQ) * C/16 of partition block q (two 16-partition replicas).
    """
    J = rows // C
    a = flat.reshape(J, 128, G).transpose(0, 2, 1).reshape(J, C)
    w = a.reshape(J, C // 16, 16).transpose(0, 2, 1)  # [J, 16, C/16]
    out = np.zeros((128, rows // 16 // NQ), np.int16)
    for j in range(J):
        q, col = j % NQ, (j // NQ) * (C // 16)
        blk = w[j].astype(np.int16)  # [16, C/16]
        out[32 * q : 32 * q + 16, col : col + C // 16] = blk
        out[32 * q + 16 : 32 * q + 32, col : col + C // 16] = blk
    return np.ascontiguousarray(out)


def host_consts(interpolation_embedding, ln_beta):
    triu = np.ascontiguousarray(np.triu(np.ones((128, 128), np.float32)))
    onesm = np.ones((128, 128), np.float32)
    n = np.arange(N, dtype=np.float32)
    alpha = (np.float32(N - 1) - n) / np.float32(N - 1)
    a3t = np.ascontiguousarray(
        np.stack([alpha, np.float32(1.0) - alpha, np.ones(N, np.float32)])
    )
    ie = np.asarray(interpolation_embedding, dtype=np.float32)
    beta = np.asarray(ln_beta, dtype=np.float32)
    i3 = np.ascontiguousarray(np.stack([ie[0], ie[1], beta]))
    return triu, onesm, a3t, i3


def _build_in_maps(inputs: dict) -> list[dict]:
    triu, onesm, a3t, i3 = host_consts(
        inputs["interpolation_embedding"], inputs["ln_beta"]
    )
    gammab = np.ascontiguousarray(
        np.tile(np.asarray(inputs["ln_gamma"], np.float32)[None, :], (128, 1))
    )
    csemb = np.ascontiguousarray(np.asarray(inputs["cumsum_embedding"], np.float32))
    flat = np.asarray(inputs["index_tensor"]).reshape(-1)
    return [
        {
            "csemb": csemb,
            "triu": triu,
            "onesm": onesm,
            "a3t": a3t,
            "i3": i3,
            "gammab": gammab,
            "idx16": prep_idx(flat[c * R : (c + 1) * R], R),
        }
        for c in range(N_CORES)
    ]


def _run(
    cumsum_embedding,
    interpolation_embedding,
    ln_gamma,
    ln_beta,
    index_tensor,
    reps: int = 1,
    **spmd_kwargs,
):
    nc = _get_nc(R, reps)
    in_maps = _build_in_maps(
        {
            "cumsum_embedding": cumsum_embedding,
            "interpolation_embedding": interpolation_embedding,
            "ln_gamma": ln_gamma,
            "ln_beta": ln_beta,
            "index_tensor": index_tensor,
        }
    )
    res = run_bass_kernel_spmd(nc, in_maps, list(range(N_CORES)), **spmd_kwargs)
    outs = [
        np.asarray(res.results[c]["out"]).reshape(B // N_CORES, L, D)
        for c in range(N_CORES)
    ]
    return np.concatenate(outs, axis=0), res


def kernel(
    cumsum_embedding,
    interpolation_embedding,
    ln_gamma,
    ln_beta,
    index_tensor,
) -> np.ndarray:
    out, _ = _run(
        cumsum_embedding, interpolation_embedding, ln_gamma, ln_beta, index_tensor
    )
    return out


# revision 5
# speedup vs baseline: 2.1871x; 2.1871x over previous
"""Trainium2 Bass kernel for nn_CumsumInterpolationEmbedding.

    E = LayerNorm(cumsum(cumsum_embedding, axis=0)) * gamma + beta
        + [alpha, 1-alpha] @ interpolation_embedding     # alpha_n = (N-1-n)/(N-1)
    out[b, l, :] = E[index_tensor[b, l], :]              # [4096, 200, 128] f32

Data-parallel over the 4096 batch across 8 NeuronCores; each core runs an
identical program on its own 1/8 of the index stream (102,400 lookups).

Per-core device program:
  prologue — build E [1024, 128] once (PE cumsum via triangular/ones matmuls,
    DVE bn_stats/bn_aggr layernorm, ACT rsqrt, K=3 matmul for interpolation +
    beta), cast it to bf16 and park it in a DRAM scratch tensor.
  main loop — J chunks of C=3200 lookups, three pipelined stages:
    * gpsimd dma_gather: 3200 descriptors of 256 B (bf16) each, table rows ->
      SBUF tile [128, G=25, 128] (row i of the chunk lands on partition i%128),
      spread round-robin over 4 SWDGE queues (the ucode max; each queue is
      served by its own gpsimd core pair).
    * upcast bf16 -> f32 on the otherwise idle DVE + ACT engines (half each).
    * HWDGE writeout of the f32 tile to the per-core contiguous output slice.

Why this shape: the gather is Q7 descriptor-EMISSION-bound (~1.8 ns/row
aggregate over 4 queues, ~185 us/pass, independent of descriptor size), and
gather + writeout share the 16-SDMA-engine pool (~360-400 GB/s). Keeping the
parked table in bf16 halves the gather's share of SDMA bytes (26 MB vs 52 MB
per pass), and the fine chunk grain (C=3200) with deep buffering (12 gather
buffers, 6 writeout buffers) overlaps the emission with the SDMA drain;
measured ~254 us/pass vs ~470 us for the f32 coarse-grained two-pass. Table
quantization to bf16 adds ~2e-3 relative error (gate is 2e-2).

The host pre-permutes each core's index stream so that partition p of chunk j
holds output rows jC + p*G .. jC + p*G + G-1; the writeout is then 128
descriptors of G*512 B contiguous DRAM each, and the gathered rows land in
exactly original output order. Host work is only casting/reordering the int
indices (no arithmetic on the data path).
"""

import contextlib

import numpy as np

import concourse.bacc as bacc
import concourse.bass as bass
import concourse.mybir as mybir
from concourse._compat import get_trn_type
from concourse.bass_utils import run_bass_kernel_spmd
from concourse.library_config import mlp

F32 = mybir.dt.float32
BF16 = mybir.dt.bfloat16
I16 = mybir.dt.int16
AF = mybir.ActivationFunctionType
ALU = mybir.AluOpType

N_CORES = 8
N = 1024            # embedding table rows
D = 128             # embedding dim
EPS = 1e-5
B, L = 4096, 200
R = B * L // N_CORES  # 102400 output rows per core
G = 25                # output rows per partition per gather chunk
C = 128 * G           # 3200 indices per gather chunk
NB = 12               # bf16 gather buffers in flight
NW = 6                # f32 upcast/writeout buffers in flight
NQ = 4                # SWDGE queues (ucode max); chunk j uses queue j % NQ
TC = N // 128         # 8 table chunks
HALF = (G * D) // 2   # upcast split point (DVE first half, ACT second)


def build_nc(rows: int = R, reps: int = 1) -> bass.Bass:
    """reps > 1 repeats the main gather/upcast/writeout loop (idempotent
    rewrites of the same output) so device time can be measured as a slope."""
    J = rows // C
    assert J * C == rows and rows % 16 == 0
    assert J % NQ == 0
    JJ = J * reps
    IXC = rows // 16 // NQ  # idx columns per queue block

    nc = bacc.Bacc(get_trn_type() or "TRN2", num_swdge_queues=NQ)
    # all inputs packed into ONE per-core blob (int16 units): a single input
    # array per core keeps the per-exec arg-marshaling cost through the axon
    # tunnel at the probe level, and the host pre-rearranged csemb makes the
    # load 128 contiguous descriptors instead of 1024
    BC = 5120 + IXC
    blob = nc.dram_tensor("blob", [128, BC], I16, kind="ExternalInput")
    out = nc.dram_tensor("out", [rows, D], F32, kind="ExternalOutput")
    etab = nc.dram_tensor("etab", [N, D], BF16)

    with contextlib.ExitStack() as ctx:
        block = ctx.enter_context(nc.Block())
        cs_sb = ctx.enter_context(nc.sbuf_tensor("cs_sb", [128, TC, D], F32))
        triu_sb = ctx.enter_context(nc.sbuf_tensor("triu_sb", [128, 128], F32))
        ones_sb = ctx.enter_context(nc.sbuf_tensor("ones_sb", [128, 128], F32))
        a3t_sb = ctx.enter_context(nc.sbuf_tensor("a3t_sb", [3, N], F32))
        i3_sb = ctx.enter_context(nc.sbuf_tensor("i3_sb", [3, D], F32))
        gamma_sb = ctx.enter_context(nc.sbuf_tensor("gamma_sb", [128, D], F32))
        e_sb = ctx.enter_context(nc.sbuf_tensor("e_sb", [128, TC, D], F32))
        eb_sb = ctx.enter_context(nc.sbuf_tensor("eb_sb", [128, TC, D], BF16))
        stats_sb = ctx.enter_context(nc.sbuf_tensor("stats_sb", [128, TC, 6], F32))
        mv_sb = ctx.enter_context(nc.sbuf_tensor("mv_sb", [128, TC, 2], F32))
        rstd_sb = ctx.enter_context(nc.sbuf_tensor("rstd_sb", [128, TC], F32))
        eps_sb = ctx.enter_context(nc.sbuf_tensor("eps_sb", [128, 1], F32))
        idx_sb = ctx.enter_context(nc.sbuf_tensor("idx_sb", [128, IXC], I16))
        gbufs = [
            ctx.enter_context(nc.sbuf_tensor(f"gbuf{b}", [128, G * D], BF16))
            for b in range(NB)
        ]
        fbufs = [
            ctx.enter_context(nc.sbuf_tensor(f"fbuf{w}", [128, G * D], F32))
            for w in range(NW)
        ]
        ps_cs = ctx.enter_context(nc.psum_tensor("ps_cs", [128, TC, D], F32))
        ps_in = ctx.enter_context(nc.psum_tensor("ps_in", [128, TC, D], F32))

        s_in = ctx.enter_context(nc.semaphore("s_in"))
        s_ix = ctx.enter_context(nc.semaphore("s_ix"))
        s_pe = ctx.enter_context(nc.semaphore("s_pe"))
        s_pi = ctx.enter_context(nc.semaphore("s_pi"))
        s_mv = ctx.enter_context(nc.semaphore("s_mv"))
        s_rs = ctx.enter_context(nc.semaphore("s_rs"))
        s_e = ctx.enter_context(nc.semaphore("s_e"))
        s_eb = ctx.enter_context(nc.semaphore("s_eb"))
        s_et = ctx.enter_context(nc.semaphore("s_et"))
        s_eps = ctx.enter_context(nc.semaphore("s_eps"))
        s_dv = ctx.enter_context(nc.semaphore("s_dv"))
        g_sems = [ctx.enter_context(nc.semaphore(f"s_g{b}")) for b in range(NB)]
        # upcast-done (per f32 buffer): DVE and ACT each inc 1 per use
        u_sems = [ctx.enter_context(nc.semaphore(f"s_u{w}")) for w in range(NW)]
        w_sems = [ctx.enter_context(nc.semaphore(f"s_w{w}")) for w in range(NW)]

        @block.sync
        def _(sp: bass.BassEngine):
            sp.dma_start(
                cs_sb[:, :, :].rearrange("p c d -> p (c d)"),
                blob[:, 0:2048].bitcast(F32),
            ).then_inc(s_in, 16)
            sp.dma_start(triu_sb[:, :], blob[:, 2048:2304].bitcast(F32)).then_inc(s_in, 16)
            sp.dma_start(ones_sb[:, :], blob[:, 2304:2560].bitcast(F32)).then_inc(s_in, 16)
            sp.dma_start(a3t_sb[:, :], blob[0:3, 2560:4608].bitcast(F32)).then_inc(s_in, 16)
            sp.dma_start(i3_sb[:, :], blob[0:3, 4608:4864].bitcast(F32)).then_inc(s_in, 16)
            sp.dma_start(gamma_sb[:, :], blob[:, 4864:5120].bitcast(F32)).then_inc(s_in, 16)
            sp.dma_start(idx_sb[:, :], blob[:, 5120 : 5120 + IXC]).then_inc(s_ix, 16)

            # park the bf16 embedding table in DRAM for the gathers
            sp.wait_ge(s_eb, 1)
            sp.dma_start(
                etab[:, :].rearrange("(c p) d -> p c d", p=128), eb_sb[:, :, :]
            ).then_inc(s_et, 16)

            # writeouts: chunk j -> output rows [jC, (j+1)C), 128 descriptors
            # of G*512 B contiguous DRAM each, from the upcast f32 buffer
            for jj in range(JJ):
                j, w, r = jj % J, jj % NW, jj // NW
                sp.wait_ge(u_sems[w], 2 * (r + 1))
                sp.dma_start(
                    out[j * C : (j + 1) * C, :].rearrange("(p g) d -> p (g d)", p=128),
                    fbufs[w][:, :],
                ).then_inc(w_sems[w], 16)
            for w in range(min(NW, JJ)):
                sp.wait_ge(w_sems[w], 16 * ((JJ - 1 - w) // NW + 1))

        @block.tensor
        def _(pe: bass.BassEngine):
            pe.wait_ge(s_in, 96)
            # TRN2 fatal hazard: PE writing a PSUM bank while DVE reads the
            # same bank crashes the core. Interleave per chunk and gate each
            # chunk's matmuls on DVE being completely done with the previous
            # chunk (s_e) so PE-writes and DVE-reads never overlap in PSUM.
            for c in range(TC):
                if c >= 1:
                    pe.wait_ge(s_e, c)
                # cumsum: chunk c rows = ones @ chunks<c + triu @ chunk c,
                # accumulated in increasing row order (exact sequential
                # prefix sum)
                for cp in range(c + 1):
                    mm = pe.matmul(
                        ps_cs[:, c, :],
                        (triu_sb if cp == c else ones_sb)[:, :],
                        cs_sb[:, cp, :],
                        start=(cp == 0),
                        stop=(cp == c),
                    )
                    if cp == c:
                        mm.then_inc(s_pe, 1)
                # interpolation + beta: [alpha, 1-alpha, 1] @ [I0; I1; beta]
                pe.matmul(
                    ps_in[:, c, :],
                    a3t_sb[:, c * 128 : (c + 1) * 128],
                    i3_sb[:, :],
                    start=True,
                    stop=True,
                ).then_inc(s_pi, 1)

        @block.scalar
        def _(act: bass.BassEngine):
            act.wait_ge(s_eps, 1)
            for c in range(TC):
                act.wait_ge(s_mv, c + 1)
                act.activation(
                    out=rstd_sb[:, c : c + 1],
                    in_=mv_sb[:, c, 1:2],
                    func=AF.Sqrt,
                    bias=eps_sb[:, 0:1],
                    scale=1.0,
                ).then_inc(s_rs, 1)
            # upcast second half of each gathered chunk (ACT share)
            for jj in range(JJ):
                j, b, w, r = jj % J, jj % NB, jj % NW, jj // NW
                act.wait_ge(g_sems[b], 16 * (jj // NB + 1))
                if r >= 1:
                    act.wait_ge(w_sems[w], 16 * r)
                act.copy(
                    out=fbufs[w][:, HALF:],
                    in_=gbufs[b][:, HALF:],
                ).then_inc(u_sems[w], 1)

        @block.vector
        def _(dve: bass.BassEngine):
            # raw-bass DVE writes are posted: serialize each dependent step
            # with a sem (prologue-only, cost is negligible)
            dv_cnt = [0]

            def step(inst):
                dv_cnt[0] += 1
                inst.then_inc(s_dv, 1)
                dve.wait_ge(s_dv, dv_cnt[0])

            dve.memset(eps_sb[:, :], EPS).then_inc(s_eps, 1)
            for c in range(TC):
                dve.wait_ge(s_pe, c + 1)
                step(dve.bn_stats(out=stats_sb[:, c, :], in_=ps_cs[:, c, :]))
                dve.bn_aggr(out=mv_sb[:, c, :], in_=stats_sb[:, c, :]).then_inc(s_mv, 1)
                dve.wait_ge(s_rs, c + 1)
                step(dve.reciprocal(out=rstd_sb[:, c : c + 1], in_=rstd_sb[:, c : c + 1]))
                step(
                    dve.tensor_scalar(
                        out=e_sb[:, c, :],
                        in0=ps_cs[:, c, :],
                        scalar1=mv_sb[:, c, 0:1],
                        scalar2=rstd_sb[:, c : c + 1],
                        op0=ALU.subtract,
                        op1=ALU.mult,
                    )
                )
                step(
                    dve.tensor_tensor(
                        out=e_sb[:, c, :],
                        in0=e_sb[:, c, :],
                        in1=gamma_sb[:, :],
                        op=ALU.mult,
                    )
                )
                dve.wait_ge(s_pi, c + 1)
                dve.tensor_tensor(
                    out=e_sb[:, c, :], in0=e_sb[:, c, :], in1=ps_in[:, c, :], op=ALU.add
                ).then_inc(s_e, 1)
            # cast the finished table to bf16 for parking
            dve.tensor_copy(
                out=eb_sb[:, :, :].rearrange("p c d -> p (c d)"),
                in_=e_sb[:, :, :].rearrange("p c d -> p (c d)"),
            ).then_inc(s_eb, 1)
            # upcast first half of each gathered chunk (DVE share)
            for jj in range(JJ):
                j, b, w, r = jj % J, jj % NB, jj % NW, jj // NW
                dve.wait_ge(g_sems[b], 16 * (jj // NB + 1))
                if r >= 1:
                    dve.wait_ge(w_sems[w], 16 * r)
                dve.tensor_copy(
                    out=fbufs[w][:, :HALF],
                    in_=gbufs[b][:, :HALF],
                ).then_inc(u_sems[w], 1)

        @block.gpsimd
        def _(gp: bass.BassGpSimd):
            gp.load_library(mlp)
            gp.wait_ge(s_ix, 16)
            gp.wait_ge(s_et, 16)
            for jj in range(JJ):
                j, b = jj % J, jj % NB
                q = j % NQ
                if jj >= NB:
                    # gbuf reusable once both upcast halves of its prior use ran
                    w_prev = (jj - NB) % NW
                    gp.wait_ge(u_sems[w_prev], 2 * ((jj - NB) // NW + 1))
                gp.dma_gather(
                    gbufs[b][:, :].rearrange("p (g d) -> p g d", d=D),
                    etab[:, :],
                    idx_sb[:, (j // NQ) * (C // 16) : (j // NQ + 1) * (C // 16)],
                    C,
                    C,
                    D,
                    # single_packet concatenates all of an engine's descriptors
                    # into one packet; at C/16=200 descriptors per engine that
                    # blows the <=64-descriptor packet ceiling and kills the
                    # core. One packet per descriptor is required here.
                    single_packet=False,
                    queue_num=q,
                ).then_inc(g_sems[b], 16)

    nc.compile()
    return nc


_NC_CACHE: dict = {}


def _get_nc(rows: int, reps: int = 1) -> bass.Bass:
    if (rows, reps) not in _NC_CACHE:
        _NC_CACHE[(rows, reps)] = build_nc(rows, reps)
    return _NC_CACHE[(rows, reps)]


def prep_idx(flat: np.ndarray, rows: int) -> np.ndarray:
    """Permute + wrap one core's index stream for dma_gather.

    Gather flat position i = g*128 + p of chunk j must hold the index for
    output row jC + p*G + g, wrapped into the ucode's [16, n/16] layout
    (position i -> partition i%16, column i//16). Chunk j is processed by
    gpsimd core pair q = j % NQ, which reads indices from partitions
    [32q, 32q+32) only; chunk j's columns are packed at column offset
    (j // NQ) * C/16 of partition block q (two 16-partition replicas).
    """
    J = rows // C
    a = flat.reshape(J, 128, G).transpose(0, 2, 1).reshape(J, C)
    w = a.reshape(J, C // 16, 16).transpose(0, 2, 1)  # [J, 16, C/16]
    out = np.zeros((128, rows // 16 // NQ), np.int16)
    for j in range(J):
        q, col = j % NQ, (j // NQ) * (C // 16)
        blk = w[j].astype(np.int16)  # [16, C/16]
        out[32 * q : 32 * q + 16, col : col + C // 16] = blk
        out[32 * q + 16 : 32 * q + 32, col : col + C // 16] = blk
    return np.ascontiguousarray(out)


def host_consts(interpolation_embedding, ln_beta):
    triu = np.ascontiguousarray(np.triu(np.ones((128, 128), np.float32)))
    onesm = np.ones((128, 128), np.float32)
    n = np.arange(N, dtype=np.float32)
    alpha = (np.float32(N - 1) - n) / np.float32(N - 1)
    a3t = np.ascontiguousarray(
        np.stack([alpha, np.float32(1.0) - alpha, np.ones(N, np.float32)])
    )
    ie = np.asarray(interpolation_embedding, dtype=np.float32)
    beta = np.asarray(ln_beta, dtype=np.float32)
    i3 = np.ascontiguousarray(np.stack([ie[0], ie[1], beta]))
    return triu, onesm, a3t, i3


def _build_in_maps(inputs: dict) -> list[dict]:
    triu, onesm, a3t, i3 = host_consts(
        inputs["interpolation_embedding"], inputs["ln_beta"]
    )
    gammab = np.ascontiguousarray(
        np.tile(np.asarray(inputs["ln_gamma"], np.float32)[None, :], (128, 1))
    )
    csemb = np.ascontiguousarray(np.asarray(inputs["cumsum_embedding"], np.float32))
    # pre-rearrange csemb to the device layout (c p) d -> p (c d)
    cs_r = np.ascontiguousarray(
        csemb.reshape(TC, 128, D).transpose(1, 0, 2).reshape(128, TC * D)
    )
    flat = np.asarray(inputs["index_tensor"]).reshape(-1)
    IXC = R // 16 // NQ

    def pack(idx16: np.ndarray) -> np.ndarray:
        blob = np.zeros((128, 5120 + IXC), np.int16)
        blob[:, 0:2048] = cs_r.view(np.int16)
        blob[:, 2048:2304] = triu.view(np.int16)
        blob[:, 2304:2560] = onesm.view(np.int16)
        blob[0:3, 2560:4608] = np.ascontiguousarray(a3t).view(np.int16)
        blob[0:3, 4608:4864] = np.ascontiguousarray(i3).view(np.int16)
        blob[:, 4864:5120] = gammab.view(np.int16)
        blob[:, 5120 : 5120 + IXC] = idx16
        return blob

    return [
        {"blob": pack(prep_idx(flat[c * R : (c + 1) * R], R))}
        for c in range(N_CORES)
    ]


def _run(
    cumsum_embedding,
    interpolation_embedding,
    ln_gamma,
    ln_beta,
    index_tensor,
    reps: int = 1,
    **spmd_kwargs,
):
    nc = _get_nc(R, reps)
    in_maps = _build_in_maps(
        {
            "cumsum_embedding": cumsum_embedding,
            "interpolation_embedding": interpolation_embedding,
            "ln_gamma": ln_gamma,
            "ln_beta": ln_beta,
            "index_tensor": index_tensor,
        }
    )
    res = run_bass_kernel_spmd(nc, in_maps, list(range(N_CORES)), **spmd_kwargs)
    outs = [
        np.asarray(res.results[c]["out"]).reshape(B // N_CORES, L, D)
        for c in range(N_CORES)
    ]
    return np.concatenate(outs, axis=0), res


def kernel(
    cumsum_embedding,
    interpolation_embedding,
    ln_gamma,
    ln_beta,
    index_tensor,
) -> np.ndarray:
    out, _ = _run(
        cumsum_embedding, interpolation_embedding, ln_gamma, ln_beta, index_tensor
    )
    return out
